# revision 2
# baseline (speedup 1.0000x reference)
"""Trainium2 kernel for nn_ContinuousLocationMap.

Contract: kernel(locs) with locs [8, 1024, 2] f32 -> [8, 2045, 2045, 4] f32.

Per batch item b the output grid is
    out[i, j] = hit(i,j) ? [1, 1, locs[b,w,0], locs[b,w,1]]
                         : [0.634, 0.634, xs[j], xs[i]]
where hit cells come from a 2x2 scatter window around each location index
and w is the last (max-ordinal) location that touched the cell.

Strategy (pure data parallel, one batch item per NeuronCore):
  host:   resolve the scatter winners (<= 4096 cells per batch item, deduped)
          and precompute the 128-row base-tile template + per-tile y columns.
  device: keep the template in SBUF, rewrite only the stride-4 y channel per
          128-row tile (cheap DVE op, hidden under DMA), stream 16 stores of
          ~4.2 MB to HBM, and patch each tile's hit cells with indirect DMAs
          gated only on that tile's store.

The output is split into 16 per-tile DRAM tensors (out0..out15) so the Tile
dependency tracker sees the per-tile scatters and the bulk stores of *other*
tiles as disjoint: scatters overlap the store stream instead of serializing
after it (indirect DMA requires AP offset 0, so slices of one big output
cannot express this). Host splits the winner cells by tile, re-bases indices
to the tile start, and pads each tile's scatter groups to full 128-partition
ops with idempotent rewrites (duplicate cells, or base-value writes for
cores with no hits in that tile). The number of scatter groups per tile is
input-dependent; the compiled program is cached per (signature, repeat).
"""

import numpy as np

# ---- hyperparameters (must mirror reference.py) ----
MIN_LOC = 0.0
MAX_LOC = 512.0
BINS = 2048
STRIDE = 1
WINDOW = 5

LOC_DELTA = (MAX_LOC - MIN_LOC) / BINS            # 0.25
WSIDE = WINDOW // 2                                # 2
BINS_WINDOW = BINS - 2 * WSIDE                     # 2044
MIN_W = MIN_LOC + LOC_DELTA * WSIDE                # 0.5
MAX_W = MIN_LOC + LOC_DELTA * BINS_WINDOW          # 511.0
G = int((BINS_WINDOW + 1) // STRIDE)               # 2045
DELTA_MAP = (MAX_W - MIN_W) / G
CORR_BASE = 0.634

BATCH = 8
N_LOCS = 1024

P = 128                       # SBUF partitions
ROWF = G * 4                  # 8180 floats per output row
NT = (G + P - 1) // P         # 16 row-tiles; last has 125 rows
TILE_ROWS = [min(P, G - t * P) for t in range(NT)]

TRACE = False                 # test.py sets this for profiling runs
LAST_RESULT = None            # BassKernelResults from the last run

_XS = (np.float32(MIN_W)
       + np.float32(DELTA_MAP) * np.arange(G, dtype=np.float32)).astype(np.float32)

_OFFS = np.array([[-1, -1], [-1, 0], [0, -1], [0, 0]], dtype=np.int32)


def _host_shared_inputs():
    """Template base tile (tile 0 content) and per-tile y columns."""
    btile = np.empty((P, ROWF), dtype=np.float32)
    btile[:, 0::4] = CORR_BASE
    btile[:, 1::4] = CORR_BASE
    btile[:, 2::4] = _XS[None, :]
    btile[:, 3::4] = _XS[:P, None]

    ycols = np.empty((P, NT), dtype=np.float32)
    for t in range(NT):
        rows = np.minimum(t * P + np.arange(P), G - 1)
        ycols[:, t] = _XS[rows]
    return btile, ycols


def _host_tile_cells(locs_b):
    """Resolve last-write-wins winners for one batch item, split by row-tile.

    Returns a list of NT (rel_cells int32 [k_t], vals f32 [k_t, 4]) pairs:
    deduped cell indices re-based to the tile's first cell, ascending, and
    the 4-float payload per cell.
    """
    locs_b = np.asarray(locs_b, dtype=np.float32)
    idx = (locs_b / np.float32(LOC_DELTA) / np.float32(STRIDE)).astype(np.int32)
    pos = idx[:, None, :] + _OFFS[None, :, :]                   # [L, 4, 2]
    valid = np.all((pos >= 0) & (pos <= G - 1), axis=-1)        # [L, 4]
    flat = np.where(valid, pos[..., 0] * G + pos[..., 1], 0)
    ordn = np.where(valid, np.arange(locs_b.shape[0], dtype=np.int64)[:, None], -1)

    cells = np.unique(flat[valid])
    winner = np.full(G * G, -1, dtype=np.int64)
    np.maximum.at(winner, flat.ravel(), ordn.ravel())
    win = winner[cells]
    keep = win >= 0
    cells, win = cells[keep], win[keep]                         # ascending

    vals = np.empty((len(cells), 4), dtype=np.float32)
    vals[:, 0] = 1.0
    vals[:, 1] = 1.0
    vals[:, 2:4] = locs_b[win]

    tiles = []
    rows = cells // G
    for t in range(NT):
        m = (rows >= t * P) & (rows < t * P + TILE_ROWS[t])
        tiles.append(((cells[m] - t * P * G).astype(np.int32), vals[m]))
    return tiles


def _signature(per_core_tiles):
    """Scatter groups per tile: max over cores of ceil(cells_t / P)."""
    return tuple(
        max(-(-len(tiles[t][0]) // P) for tiles in per_core_tiles)
        for t in range(NT)
    )


def _base_payload(t):
    """The base (miss) value at the first cell of tile t — an idempotent
    pad write for cores with no hits in that tile."""
    return np.array([CORR_BASE, CORR_BASE, _XS[0], _XS[t * P]], dtype=np.float32)


def _host_pack(tiles_b, sig):
    """Pack one core's per-tile cells into (hidx [P, S], hval [P, 4*S]) with
    S = sum(sig) groups laid out tile-major; pad slots repeat a real cell of
    the tile (idempotent) or write the base value at the tile's cell 0."""
    S = sum(sig)
    hidx = np.empty((S, P), dtype=np.int32)
    hval = np.empty((S, P, 4), dtype=np.float32)
    col = 0
    for t, n_groups in enumerate(sig):
        rel, vals = tiles_b[t]
        k = len(rel)
        for gindex in range(n_groups):
            lo = gindex * P
            sl_idx = hidx[col]
            sl_val = hval[col]
            take = max(0, min(P, k - lo))
            if take:
                sl_idx[:take] = rel[lo:lo + take]
                sl_val[:take] = vals[lo:lo + take]
            if take < P:
                if k:
                    sl_idx[take:] = rel[k - 1]
                    sl_val[take:] = vals[k - 1]
                else:
                    sl_idx[take:] = 0
                    sl_val[take:] = _base_payload(t)
            col += 1
    return hidx.T.copy(), hval.transpose(1, 0, 2).reshape(P, S * 4).copy()


_NC_CACHE = {}


def _build_nc(repeat=1, sig=None):
    """Build the per-core Bass program (same program on all 8 cores).

    repeat>1 unrolls the whole store+scatter pipeline N times inside one
    NEFF (idempotent rewrites) — used by the bench to isolate steady-state
    device time from the ~340ms per-call PJRT/axon dispatch overhead.
    """
    from concourse import bass, bacc, mybir
    import concourse.tile as tile
    from concourse.tile import add_dep_helper

    if sig is None:
        sig = (2,) * NT
    S = sum(sig)

    nc = bacc.Bacc(None, target_bir_lowering=False)
    f32 = mybir.dt.float32
    btile = nc.dram_tensor("btile", [P, ROWF], f32, kind="ExternalInput")
    ycols = nc.dram_tensor("ycols", [P, NT], f32, kind="ExternalInput")
    hidx = nc.dram_tensor("hidx", [P, S], mybir.dt.int32, kind="ExternalInput")
    hval = nc.dram_tensor("hval", [P, S * 4], f32, kind="ExternalInput")
    outs = [
        nc.dram_tensor(f"out{t}", [TILE_ROWS[t] * G, 4], f32,
                       kind="ExternalOutput")
        for t in range(NT)
    ]
    out_rows = [o[:].rearrange("(g w) c -> g (w c)", w=G) for o in outs]

    with tile.TileContext(nc) as tc:
        with tc.tile_pool(name="big", bufs=1) as big, \
             tc.tile_pool(name="small", bufs=1) as small:
            yc = small.tile([P, NT], f32, tag="yc")
            hi = small.tile([P, S], mybir.dt.int32, tag="hi")
            hv = small.tile([P, S * 4], f32, tag="hv")
            nc.sync.dma_start(out=yc[:], in_=ycols[:])
            nc.sync.dma_start(out=hi[:], in_=hidx[:])
            nc.sync.dma_start(out=hv[:], in_=hval[:])

            buf_a = big.tile([P, ROWF], f32, tag="bufA")
            buf_b = big.tile([P, ROWF], f32, tag="bufB")
            bufs = [buf_a, buf_b]
            nc.sync.dma_start(out=bufs[0][:], in_=btile[:])
            nc.vector.tensor_copy(out=bufs[1][:], in_=bufs[0][:])

            for rep in range(repeat):
                col = 0
                for t in range(NT):
                    buf = bufs[t % 2]
                    rows = TILE_ROWS[t]
                    if t >= 1 or rep > 0:  # template holds tile 0's y channel
                        nc.vector.tensor_copy(
                            out=buf[:, 3::4],
                            in_=yc[:, t:t + 1].to_broadcast([P, G]),
                        )
                    st = nc.sync.dma_start(
                        out=out_rows[t][:rows, :],
                        in_=buf[:rows, :],
                    )
                    # HW DGE consumes ONE offset per partition and streams
                    # that partition's whole in_ free dim contiguously — so
                    # each op scatters 128 cells (idx [128,1], payload
                    # [128,4]) into this tile's own output tensor. Gated on
                    # this tile's store only; stores of later tiles touch
                    # different tensors and stream on unimpeded.
                    for _ in range(sig[t]):
                        sc = nc.gpsimd.indirect_dma_start(
                            out=outs[t][:],
                            out_offset=bass.IndirectOffsetOnAxis(
                                ap=hi[:, col:col + 1], axis=0),
                            in_=hv[:, 4 * col:4 * col + 4],
                            in_offset=None,
                        )
                        add_dep_helper(sc.ins, st.ins)
                        col += 1
    nc.finalize()
    return nc


def kernel(locs):
    global LAST_RESULT
    from concourse.bass_utils import run_bass_kernel_spmd

    locs = np.asarray(locs, dtype=np.float32)
    assert locs.shape == (BATCH, N_LOCS, 2)

    btile, ycols = _host_shared_inputs()
    per_core_tiles = [_host_tile_cells(locs[b]) for b in range(BATCH)]
    sig = _signature(per_core_tiles)

    in_maps = []
    for b in range(BATCH):
        hidx, hval = _host_pack(per_core_tiles[b], sig)
        in_maps.append({"btile": btile, "ycols": ycols,
                        "hidx": hidx, "hval": hval})

    key = (sig, 1)
    if key not in _NC_CACHE:
        _NC_CACHE.clear()
        _NC_CACHE[key] = _build_nc(repeat=1, sig=sig)
    nc = _NC_CACHE[key]

    res = run_bass_kernel_spmd(nc, in_maps, core_ids=list(range(BATCH)),
                               trace=TRACE)
    LAST_RESULT = res
    out = np.empty((BATCH, G * G, 4), dtype=np.float32)
    for b in range(BATCH):
        row0 = 0
        for t in range(NT):
            n = TILE_ROWS[t] * G
            out[b, row0:row0 + n] = res.results[b][f"out{t}"]
            row0 += n
    return out.reshape(BATCH, G, G, 4)


# revision 4
# speedup vs baseline: 3.7636x; 3.7636x over previous
"""Trainium2 kernel for nn_ContinuousLocationMap.

Contract: kernel(locs) with locs [8, 1024, 2] f32 -> [8, 2045, 2045, 4] f32.

Per batch item b the output grid is
    out[i, j] = hit(i,j) ? [1, 1, locs[b,w,0], locs[b,w,1]]
                         : [0.634, 0.634, xs[j], xs[i]]
where hit cells come from a 2x2 scatter window around each location index
and w is the last (max-ordinal) location that touched the cell.

Strategy (pure data parallel, one batch item per NeuronCore):
  host:   resolve the scatter winners (<= 4096 cells per batch item, deduped)
          and merge them into j-adjacent PAIRS (each 2x2 window writes two
          j-adjacent cells per row, so hits come in runs >= 2; lone leftovers
          are extended with the host-known final value of a neighbor cell).
  device: keep the template in SBUF, rewrite only the stride-4 y channel per
          128-row tile (cheap DVE op, hidden under DMA), stream 16 stores of
          ~4.2 MB to HBM, then patch hit cells with ~16 indirect DMAs of 128
          pair-writes (32B each) in the post-store tail.

The HW DGE consumes ONE offset per partition per op and streams that
partition's whole in_ free dim contiguously from it, so an op's unit is 128
runs; merging cells into 32B pair-runs halves the op count vs per-cell
writes (~37 us tail instead of ~74 us). Scatters must stay in the tail:
concurrent with the saturated store stream each SWDGE op costs ~12.5 us
(HBM completion receipt + descriptor-ring port contention under load).
"""

import numpy as np

# ---- hyperparameters (must mirror reference.py) ----
MIN_LOC = 0.0
MAX_LOC = 512.0
BINS = 2048
STRIDE = 1
WINDOW = 5

LOC_DELTA = (MAX_LOC - MIN_LOC) / BINS            # 0.25
WSIDE = WINDOW // 2                                # 2
BINS_WINDOW = BINS - 2 * WSIDE                     # 2044
MIN_W = MIN_LOC + LOC_DELTA * WSIDE                # 0.5
MAX_W = MIN_LOC + LOC_DELTA * BINS_WINDOW          # 511.0
G = int((BINS_WINDOW + 1) // STRIDE)               # 2045
DELTA_MAP = (MAX_W - MIN_W) / G
CORR_BASE = 0.634

BATCH = 8
N_LOCS = 1024

P = 128                       # SBUF partitions
ROWF = G * 4                  # 8180 floats per output row
NT = (G + P - 1) // P         # 16 row-tiles; last has 125 rows
PCOLS = 16                    # pair-scatter ops (128 pairs each) per rep

TRACE = False                 # test.py sets this for profiling runs
LAST_RESULT = None            # BassKernelResults from the last run

_XS = (np.float32(MIN_W)
       + np.float32(DELTA_MAP) * np.arange(G, dtype=np.float32)).astype(np.float32)

_OFFS = np.array([[-1, -1], [-1, 0], [0, -1], [0, 0]], dtype=np.int32)


def _host_shared_inputs():
    """Template base tile (tile 0 content) and per-tile y columns."""
    btile = np.empty((P, ROWF), dtype=np.float32)
    btile[:, 0::4] = CORR_BASE
    btile[:, 1::4] = CORR_BASE
    btile[:, 2::4] = _XS[None, :]
    btile[:, 3::4] = _XS[:P, None]

    ycols = np.empty((P, NT), dtype=np.float32)
    for t in range(NT):
        rows = np.minimum(t * P + np.arange(P), G - 1)
        ycols[:, t] = _XS[rows]
    return btile, ycols


def _base_val(cell):
    """The base (miss) 4-vector at a flat cell — bit-identical to the
    bulk-store template, so rewriting it is idempotent."""
    return np.array([CORR_BASE, CORR_BASE, _XS[cell % G], _XS[cell // G]],
                    dtype=np.float32)


def _host_scatter(locs_b, pcols=PCOLS):
    """Resolve last-write-wins winners and pack them as 32B pair-writes.

    Returns (hidx [P, pcols] int32, hval [P, pcols*8] f32) — pair-start cell
    indices into the [G*G, 4] output and 8-float payloads (two cells), padded
    to pcols*128 pairs with duplicates (idempotent) — or None if the input
    needs more than pcols*128 pairs (caller rebuilds with a bigger program).
    """
    locs_b = np.asarray(locs_b, dtype=np.float32)
    idx = (locs_b / np.float32(LOC_DELTA) / np.float32(STRIDE)).astype(np.int32)
    pos = idx[:, None, :] + _OFFS[None, :, :]                   # [L, 4, 2]
    valid = np.all((pos >= 0) & (pos <= G - 1), axis=-1)        # [L, 4]
    flat = np.where(valid, pos[..., 0] * G + pos[..., 1], 0)
    ordn = np.where(valid, np.arange(locs_b.shape[0], dtype=np.int64)[:, None], -1)

    cells = np.unique(flat[valid])
    winner = np.full(G * G, -1, dtype=np.int64)
    np.maximum.at(winner, flat.ravel(), ordn.ravel())
    win = winner[cells]
    keep = win >= 0
    cells, win = cells[keep], win[keep]                         # ascending
    k = len(cells)

    vals = np.empty((k, 4), dtype=np.float32)
    vals[:, 0] = 1.0
    vals[:, 1] = 1.0
    vals[:, 2:4] = locs_b[win]

    pidx = []
    pval = []
    i = 0
    while i < k:
        c = int(cells[i])
        if i + 1 < k and cells[i + 1] == c + 1 and (c + 1) % G != 0:
            pidx.append(c)
            pval.append(np.concatenate([vals[i], vals[i + 1]]))
            i += 2
        elif (c + 1) % G != 0:
            # lone hit, room to the right: c+1 is not hit (it would have
            # paired) — write its base value, identical to the bulk store.
            pidx.append(c)
            pval.append(np.concatenate([vals[i], _base_val(c + 1)]))
            i += 1
        else:
            # row end: extend left; c-1 is either the (already written)
            # previous hit — rewrite its hit value — or a base cell.
            left = (vals[i - 1] if i > 0 and cells[i - 1] == c - 1
                    else _base_val(c - 1))
            pidx.append(c - 1)
            pval.append(np.concatenate([left, vals[i]]))
            i += 1

    npairs = len(pidx)
    if npairs > pcols * P:
        return None
    hidx = np.empty(pcols * P, dtype=np.int32)
    hval = np.empty((pcols * P, 8), dtype=np.float32)
    hidx[:npairs] = pidx
    hval[:npairs] = pval
    hidx[npairs:] = pidx[-1]               # idempotent duplicate writes
    hval[npairs:] = pval[-1]
    # entry (p, g) of op g is (hidx[p, g], hval[p, 8g:8g+8]).
    return (hidx.reshape(pcols, P).T.copy(),
            hval.reshape(pcols, P, 8).transpose(1, 0, 2).reshape(P, pcols * 8).copy())


_NC_CACHE = {}


def _build_nc(repeat=1, pcols=PCOLS):
    """Build the per-core Bass program (same program on all 8 cores).

    repeat>1 unrolls the whole store+scatter pipeline N times inside one
    NEFF (idempotent rewrites) — used by the bench to isolate steady-state
    device time from the ~340ms per-call PJRT/axon dispatch overhead.
    """
    from concourse import bass, bacc, mybir
    import concourse.tile as tile
    from concourse.tile import add_dep_helper

    nc = bacc.Bacc(None, target_bir_lowering=False)
    f32 = mybir.dt.float32
    btile = nc.dram_tensor("btile", [P, ROWF], f32, kind="ExternalInput")
    ycols = nc.dram_tensor("ycols", [P, NT], f32, kind="ExternalInput")
    hidx = nc.dram_tensor("hidx", [P, pcols], mybir.dt.int32, kind="ExternalInput")
    hval = nc.dram_tensor("hval", [P, pcols * 8], f32, kind="ExternalInput")
    out = nc.dram_tensor("out", [G * G, 4], f32, kind="ExternalOutput")
    out_rows = out[:].rearrange("(g w) c -> g (w c)", w=G)      # [G, ROWF]

    with tile.TileContext(nc) as tc:
        with tc.tile_pool(name="big", bufs=1) as big, \
             tc.tile_pool(name="small", bufs=1) as small:
            yc = small.tile([P, NT], f32, tag="yc")
            hi = small.tile([P, pcols], mybir.dt.int32, tag="hi")
            hv = small.tile([P, pcols * 8], f32, tag="hv")
            nc.sync.dma_start(out=yc[:], in_=ycols[:])
            nc.sync.dma_start(out=hi[:], in_=hidx[:])
            nc.sync.dma_start(out=hv[:], in_=hval[:])

            buf_a = big.tile([P, ROWF], f32, tag="bufA")
            buf_b = big.tile([P, ROWF], f32, tag="bufB")
            bufs = [buf_a, buf_b]
            nc.sync.dma_start(out=bufs[0][:], in_=btile[:])
            nc.vector.tensor_copy(out=bufs[1][:], in_=bufs[0][:])

            for rep in range(repeat):
                stores = []
                for t in range(NT):
                    buf = bufs[t % 2]
                    rows = min(P, G - t * P)
                    if t >= 1 or rep > 0:  # template holds tile 0's y channel
                        nc.vector.tensor_copy(
                            out=buf[:, 3::4],
                            in_=yc[:, t:t + 1].to_broadcast([P, G]),
                        )
                    st = nc.sync.dma_start(
                        out=out_rows[t * P:t * P + rows, :],
                        in_=buf[:rows, :],
                    )
                    stores.append(st)

                # HW DGE consumes ONE offset per partition and streams that
                # partition's whole in_ free dim contiguously from it — so
                # each op scatters 128 pairs (idx [128,1], payload [128,8]).
                for g in range(pcols):
                    sc = nc.gpsimd.indirect_dma_start(
                        out=out[:],
                        out_offset=bass.IndirectOffsetOnAxis(
                            ap=hi[:, g:g + 1], axis=0),
                        in_=hv[:, 8 * g:8 * g + 8],
                        in_offset=None,
                    )
                    # Must land after every base-tile store (Tile also tracks
                    # this conservatively; keep it explicit for safety).
                    add_dep_helper(sc.ins, stores[-1].ins)
    nc.finalize()
    return nc


def kernel(locs):
    global LAST_RESULT
    from concourse.bass_utils import run_bass_kernel_spmd

    locs = np.asarray(locs, dtype=np.float32)
    assert locs.shape == (BATCH, N_LOCS, 2)

    btile, ycols = _host_shared_inputs()
    pcols = PCOLS
    while True:
        scat = [_host_scatter(locs[b], pcols) for b in range(BATCH)]
        if all(s is not None for s in scat):
            break
        pcols *= 2              # dense-overlap inputs: bigger scatter program
    in_maps = [{"btile": btile, "ycols": ycols, "hidx": s[0], "hval": s[1]}
               for s in scat]

    if pcols not in _NC_CACHE:
        _NC_CACHE.clear()
        _NC_CACHE[pcols] = _build_nc(pcols=pcols)
    nc = _NC_CACHE[pcols]

    res = run_bass_kernel_spmd(nc, in_maps, core_ids=list(range(BATCH)),
                               trace=TRACE)
    LAST_RESULT = res
    outs = [res.results[b]["out"].reshape(G, G, 4) for b in range(BATCH)]
    return np.stack(outs, axis=0)
